# revision 5
# baseline (speedup 1.0000x reference)
"""Causal Conv1d (B=8, C=256, T=4096, H=512, K=4) on 8 TRN2 NeuronCores.

Strategy: data-parallel over batch — core i computes batch i.
Per core: out[h, t] = sum_{k, c} W[h, c*K+k] * xpad[c, t+k] + bias[h]
where xpad is x left-padded by K-1 zeros (host side).

The conv is expressed as 8 accumulating PE matmuls per [128h x 512t]
output tile: contraction dim = 128 c-channels, one matmul per
(c_chunk in 2) x (tap k in 4), with the rhs being a shifted slice of a
[128, 512+3] SBUF x tile. Weights are host-transposed to lhsT layout
[c, h] and kept SBUF-resident. Inputs are bitcast to float32r for
full-rate PE streaming (1 cycle/row); accumulation is fp32 in PSUM.
Bias is fused into the PSUM->SBUF copy (DVE tensor_scalar_add).
"""

import numpy as np

import concourse.bass as bass
import concourse.mybir as mybir
import concourse.tile as tile
from concourse import bacc
from concourse import bass2jax

B, C, T = 8, 256, 4096
H, K = 512, 4
PAD = K - 1

N_CORES = 8
TT = 512                # t-tile (free dim per matmul, one fp32 PSUM bank)
N_TTILES = T // TT      # 8
N_HCHUNK = H // 128     # 4
N_CCHUNK = C // 128     # 2
N_MM = N_CCHUNK * K     # 8 accumulating matmuls per output tile

_COMPILED = {}


def _build():
    f32 = mybir.dt.float32
    f32r = mybir.dt.float32r
    nc = bacc.Bacc("TRN2", target_bir_lowering=False, debug=False)

    x_ext = nc.declare_dram_parameter("x", [C, T + PAD], f32, isOutput=False)
    # wt[q] = lhsT tile for (k, c_chunk) with q = k*N_CCHUNK + cc: [128c, 512h]
    wt_ext = nc.declare_dram_parameter("wt", [N_MM, 128, H], f32, isOutput=False)
    # bias_mat[p, j] = b[j*128 + p]
    b_ext = nc.declare_dram_parameter("bias", [128, N_HCHUNK], f32, isOutput=False)
    out_ext = nc.declare_dram_parameter("out", [H, T], f32, isOutput=True)

    with tile.TileContext(nc) as tc:
        with (
            tc.tile_pool(name="wpool", bufs=1) as wpool,
            tc.tile_pool(name="xpool", bufs=3) as xpool,
            tc.tile_pool(name="opool", bufs=4) as opool,
            tc.tile_pool(name="psum", bufs=8, space="PSUM") as psum_pool,
        ):
            wstage = wpool.tile([128, N_MM * H], f32)
            for q in range(N_MM):
                nc.sync.dma_start(wstage[:, q * H : (q + 1) * H], wt_ext[q])
            # fp32r (tf32) matmul operands must be produced by a rounding
            # compute op — DVE copies f32 -> f32r.
            wtile_r = wpool.tile([128, N_MM * H], f32r)
            for half in range(2):
                sl = slice(half * N_MM * H // 2, (half + 1) * N_MM * H // 2)
                nc.vector.tensor_copy(wtile_r[:, sl], wstage[:, sl])
            btile = wpool.tile([128, N_HCHUNK], f32)
            nc.sync.dma_start(btile[:], b_ext[:])

            for ti in range(N_TTILES):
                xts = []
                for cc in range(N_CCHUNK):
                    xt = xpool.tile([128, TT + PAD], f32, name=f"xt{cc}", tag=f"xt{cc}")
                    nc.sync.dma_start(
                        xt[:], x_ext[cc * 128 : (cc + 1) * 128, ti * TT : ti * TT + TT + PAD]
                    )
                    xr = xpool.tile(
                        [128, TT + PAD], f32r, name=f"xr{cc}", tag=f"xr{cc}"
                    )
                    nc.vector.tensor_copy(xr[:], xt[:])
                    xts.append(xr)
                for hj in range(N_HCHUNK):
                    ps = psum_pool.tile([128, TT], f32, name="ps", tag="ps")
                    for q in range(N_MM):
                        k, cc = divmod(q, N_CCHUNK)
                        nc.tensor.matmul(
                            ps[:],
                            wtile_r[:, q * H + hj * 128 : q * H + hj * 128 + 128],
                            xts[cc][:, k : k + TT],
                            start=(q == 0),
                            stop=(q == N_MM - 1),
                        )
                    ot = opool.tile([128, TT], f32, name="ot", tag="ot")
                    nc.vector.tensor_scalar_add(ot[:], ps[:], btile[:, hj : hj + 1])
                    nc.sync.dma_start(
                        out_ext[hj * 128 : (hj + 1) * 128, ti * TT : (ti + 1) * TT],
                        ot[:],
                    )

    nc.compile()
    return nc


def get_nc():
    if "nc" not in _COMPILED:
        _COMPILED["nc"] = _build()
    return _COMPILED["nc"]


def _prep_inputs(x, W, b):
    x = np.asarray(x, dtype=np.float32)
    W = np.asarray(W, dtype=np.float32)
    b = np.asarray(b, dtype=np.float32)

    xpad = np.zeros((B, C, T + PAD), dtype=np.float32)
    xpad[:, :, PAD:] = x

    kern = W.reshape(H, C, K)
    wt = np.empty((N_MM, 128, H), dtype=np.float32)
    for k in range(K):
        for cc in range(N_CCHUNK):
            q = k * N_CCHUNK + cc
            wt[q] = np.ascontiguousarray(kern[:, cc * 128 : (cc + 1) * 128, k].T)

    bias_mat = np.ascontiguousarray(b.reshape(N_HCHUNK, 128).T)
    return xpad, wt, bias_mat


def _get_exec():
    """Build (once) a jitted shard_map executable over the 8 cores.

    Mirrors bass2jax.run_bass_via_pjrt but caches the compiled callable so
    repeated runs (timing loops) don't re-trace / re-compile.
    """
    if "exec" in _COMPILED:
        return _COMPILED["exec"]

    import jax
    from jax.experimental.shard_map import shard_map
    from jax.sharding import Mesh, PartitionSpec

    nc = get_nc()
    bass2jax.install_neuronx_cc_hook()
    assert nc.dbg_addr is None
    partition_name = nc.partition_id_tensor.name if nc.partition_id_tensor else None

    in_names, out_names, out_avals, zero_outs = [], [], [], []
    for alloc in nc.m.functions[0].allocations:
        if not isinstance(alloc, mybir.MemoryLocationSet):
            continue
        name = alloc.memorylocations[0].name
        if alloc.kind == "ExternalInput":
            if name != partition_name:
                in_names.append(name)
        elif alloc.kind == "ExternalOutput":
            shape = tuple(alloc.tensor_shape)
            dtype = mybir.dt.np(alloc.dtype)
            out_names.append(name)
            out_avals.append(jax.core.ShapedArray(shape, dtype))
            zero_outs.append(np.zeros(shape, dtype))
    n_params = len(in_names)
    all_names = in_names + out_names
    if partition_name is not None:
        all_names = all_names + [partition_name]

    def _body(*args):
        operands = list(args)
        if partition_name is not None:
            operands.append(bass2jax.partition_id_tensor())
        outs = bass2jax._bass_exec_p.bind(
            *operands,
            out_avals=tuple(out_avals),
            in_names=tuple(all_names),
            out_names=tuple(out_names),
            lowering_input_output_aliases=(),
            sim_require_finite=True,
            sim_require_nnan=True,
            nc=nc,
        )
        return tuple(outs)

    devices = jax.devices()[:N_CORES]
    mesh = Mesh(np.asarray(devices), ("core",))
    n_args = n_params + len(out_names)
    sharded = jax.jit(
        shard_map(
            _body,
            mesh=mesh,
            in_specs=(PartitionSpec("core"),) * n_args,
            out_specs=(PartitionSpec("core"),) * len(out_names),
            check_rep=False,
        ),
        keep_unused=True,
    )
    _COMPILED["exec"] = (sharded, in_names, out_names, out_avals, zero_outs, mesh)
    return _COMPILED["exec"]


def _make_args(in_maps):
    sharded, in_names, out_names, out_avals, zero_outs, mesh = _get_exec()
    concat_in = [
        np.concatenate([np.asarray(in_maps[c][nm]) for c in range(N_CORES)], axis=0)
        for nm in in_names
    ]
    concat_zeros = [
        np.zeros((N_CORES * z.shape[0], *z.shape[1:]), z.dtype) for z in zero_outs
    ]
    return concat_in + concat_zeros


def _run(in_maps):
    sharded, in_names, out_names, out_avals, zero_outs, mesh = _get_exec()
    out_arrs = sharded(*_make_args(in_maps))
    return [
        {
            nm: np.asarray(out_arrs[i]).reshape(N_CORES, *out_avals[i].shape)[c]
            for i, nm in enumerate(out_names)
        }
        for c in range(N_CORES)
    ]


def make_in_maps(x, W, b):
    xpad, wt, bias_mat = _prep_inputs(x, W, b)
    return [
        {"x": np.ascontiguousarray(xpad[i]), "wt": wt, "bias": bias_mat}
        for i in range(N_CORES)
    ]


def kernel(x, W, b):
    results = _run(make_in_maps(x, W, b))
    return np.stack([results[i]["out"] for i in range(N_CORES)], axis=0)


# revision 22
# speedup vs baseline: 7.3958x; 7.3958x over previous
"""Causal Conv1d (B=8, C=256, T=4096, H=512, K=4) on 8 TRN2 NeuronCores.

Strategy: data-parallel over batch — core i computes batch i.
Per core: out[h, t] = sum_{k, c} W[h, c*K+k] * xpad[c, t+k] + bias[h]
where xpad is x left-padded by K-1 zeros (host side).

The conv is expressed as 8 accumulating PE matmuls per [128h x 512t]
output tile: contraction dim = 128 c-channels, one matmul per
(c_chunk in 2) x (tap k in 4), with the rhs being a shifted slice of a
[128, 512+3] SBUF x tile. Weights are host-transposed to lhsT layout
[c, h] and kept SBUF-resident. Inputs are bitcast to float32r for
full-rate PE streaming (1 cycle/row); accumulation is fp32 in PSUM.
Bias is fused into the PSUM->SBUF copy (DVE tensor_scalar_add).
"""

import numpy as np

import concourse.bass as bass
import concourse.mybir as mybir
import concourse.tile as tile
from concourse import bacc
from concourse import bass2jax

B, C, T = 8, 256, 4096
H, K = 512, 4
PAD = K - 1

N_CORES = 8
TT = 512                # t-tile (free dim per matmul, one fp32 PSUM bank)
N_TTILES = T // TT      # 8
N_HCHUNK = H // 128     # 4
N_CCHUNK = C // 128     # 2
N_MM = N_CCHUNK * K     # 8 accumulating matmuls per output tile

_COMPILED = {}


def _build(reps=1, bias_engine="vector", xbufs=3, obufs=4, psbufs=8, order="ti"):
    f32 = mybir.dt.float32
    f32r = mybir.dt.float32r
    nc = bacc.Bacc("TRN2", target_bir_lowering=False, debug=False)

    # x/wt hold host-side tf32-rounded data; declaring them fp32r lets the
    # matmul consume DMA'd tiles directly (no on-chip rounding pass).
    x_ext = nc.declare_dram_parameter("x", [C, T + PAD], f32r, isOutput=False)
    # wt[hj][c, q*128+m]: lhsT for (q=k*N_CCHUNK+cc, h-chunk hj) — chunked by
    # hj so the first psum group only waits on a 0.5 MB load.
    wt_ext = nc.declare_dram_parameter(
        "wt", [N_HCHUNK, 128, N_MM * 128], f32r, isOutput=False
    )
    # bias_mat[p, j] = b[j*128 + p]
    b_ext = nc.declare_dram_parameter("bias", [128, N_HCHUNK], f32, isOutput=False)
    out_ext = nc.declare_dram_parameter("out", [H, T], f32, isOutput=True)

    with tile.TileContext(nc) as tc:
        with (
            tc.tile_pool(name="wpool", bufs=1) as wpool,
            tc.tile_pool(name="xpool", bufs=xbufs) as xpool,
            tc.tile_pool(name="opool", bufs=obufs) as opool,
            tc.tile_pool(name="psum", bufs=psbufs, space="PSUM") as psum_pool,
        ):

            CH = N_MM * 128  # per-h-chunk weight columns

            def body():
                wtile_r = wpool.tile([128, N_HCHUNK * CH], f32r, name="wtile_r")
                for hj in range(N_HCHUNK):
                    nc.sync.dma_start(wtile_r[:, hj * CH : (hj + 1) * CH], wt_ext[hj])
                btile = wpool.tile([128, N_HCHUNK], f32, name="btile")
                nc.sync.dma_start(btile[:], b_ext[:])

                def emit_group(ti, hj, xts):
                    ps = psum_pool.tile([128, TT], f32, name="ps", tag="ps")
                    for q in range(N_MM):
                        k, cc = divmod(q, N_CCHUNK)
                        nc.tensor.matmul(
                            ps[:],
                            wtile_r[:, hj * CH + q * 128 : hj * CH + q * 128 + 128],
                            xts[cc][:, k : k + TT],
                            start=(q == 0),
                            stop=(q == N_MM - 1),
                        )
                    ot = opool.tile([128, TT], f32, name="ot", tag="ot")
                    if bias_engine == "scalar":
                        nc.scalar.add(ot[:], ps[:], btile[:, hj : hj + 1])
                    elif bias_engine == "both":
                        if hj % 2:
                            nc.scalar.add(ot[:], ps[:], btile[:, hj : hj + 1])
                        else:
                            nc.vector.tensor_scalar_add(
                                ot[:], ps[:], btile[:, hj : hj + 1]
                            )
                    else:
                        nc.vector.tensor_scalar_add(ot[:], ps[:], btile[:, hj : hj + 1])
                    nc.sync.dma_start(
                        out_ext[hj * 128 : (hj + 1) * 128, ti * TT : (ti + 1) * TT],
                        ot[:],
                    )

                def load_x(ti, cc, tag=None, bufs=None):
                    xr = xpool.tile(
                        [128, TT + PAD],
                        f32r,
                        name=f"xr{cc}_{ti}",
                        tag=tag or f"xr{cc}",
                        **({"bufs": bufs} if bufs else {}),
                    )
                    nc.sync.dma_start(
                        xr[:],
                        x_ext[cc * 128 : (cc + 1) * 128, ti * TT : ti * TT + TT + PAD],
                    )
                    return xr

                if order == "ti":
                    for ti in range(N_TTILES):
                        xts = [load_x(ti, cc) for cc in range(N_CCHUNK)]
                        for hj in range(N_HCHUNK):
                            emit_group(ti, hj, xts)
                else:  # order == "hj": W chunks stream in; all x tiles resident
                    all_x = [
                        [
                            load_x(ti, cc, tag=f"xr{cc}_{ti}", bufs=1)
                            for cc in range(N_CCHUNK)
                        ]
                        for ti in range(N_TTILES)
                    ]
                    for hj in range(N_HCHUNK):
                        for ti in range(N_TTILES):
                            emit_group(ti, hj, all_x[ti])

            if reps == 1:
                body()
            else:
                with tc.For_i(0, reps, 1):
                    body()

    nc.compile()
    return nc


def get_nc():
    if "nc" not in _COMPILED:
        _COMPILED["nc"] = _build()
    return _COMPILED["nc"]


def _tf32_round(a):
    """Round fp32 to tf32 (10-bit mantissa) with round-to-nearest-even."""
    u = np.ascontiguousarray(a, dtype=np.float32).view(np.uint32)
    lsb = (u >> np.uint32(13)) & np.uint32(1)
    u = u + np.uint32(0x0FFF) + lsb
    u &= np.uint32(0xFFFFE000)
    return u.view(np.float32)


def _prep_inputs(x, W, b):
    x = _tf32_round(np.asarray(x, dtype=np.float32))
    W = _tf32_round(np.asarray(W, dtype=np.float32))
    b = np.asarray(b, dtype=np.float32)

    xpad = np.zeros((B, C, T + PAD), dtype=np.float32)
    xpad[:, :, PAD:] = x

    kern = W.reshape(H, C, K)
    wt = np.empty((N_HCHUNK, 128, N_MM * 128), dtype=np.float32)
    for hj in range(N_HCHUNK):
        for k in range(K):
            for cc in range(N_CCHUNK):
                q = k * N_CCHUNK + cc
                wt[hj, :, q * 128 : (q + 1) * 128] = kern[
                    hj * 128 : (hj + 1) * 128, cc * 128 : (cc + 1) * 128, k
                ].T

    bias_mat = np.ascontiguousarray(b.reshape(N_HCHUNK, 128).T)
    return xpad, wt, bias_mat


def _get_exec():
    """Build (once) a jitted shard_map executable over the 8 cores.

    Mirrors bass2jax.run_bass_via_pjrt but caches the compiled callable so
    repeated runs (timing loops) don't re-trace / re-compile.
    """
    if "exec" in _COMPILED:
        return _COMPILED["exec"]

    import jax
    from jax.experimental.shard_map import shard_map
    from jax.sharding import Mesh, PartitionSpec

    nc = get_nc()
    bass2jax.install_neuronx_cc_hook()
    assert nc.dbg_addr is None
    partition_name = nc.partition_id_tensor.name if nc.partition_id_tensor else None

    in_names, out_names, out_avals, zero_outs = [], [], [], []
    for alloc in nc.m.functions[0].allocations:
        if not isinstance(alloc, mybir.MemoryLocationSet):
            continue
        name = alloc.memorylocations[0].name
        if alloc.kind == "ExternalInput":
            if name != partition_name:
                in_names.append(name)
        elif alloc.kind == "ExternalOutput":
            shape = tuple(alloc.tensor_shape)
            dtype = mybir.dt.np(alloc.dtype)
            out_names.append(name)
            out_avals.append(jax.core.ShapedArray(shape, dtype))
            zero_outs.append(np.zeros(shape, dtype))
    n_params = len(in_names)
    all_names = in_names + out_names
    if partition_name is not None:
        all_names = all_names + [partition_name]

    def _body(*args):
        operands = list(args)
        if partition_name is not None:
            operands.append(bass2jax.partition_id_tensor())
        outs = bass2jax._bass_exec_p.bind(
            *operands,
            out_avals=tuple(out_avals),
            in_names=tuple(all_names),
            out_names=tuple(out_names),
            lowering_input_output_aliases=(),
            sim_require_finite=True,
            sim_require_nnan=True,
            nc=nc,
        )
        return tuple(outs)

    devices = jax.devices()[:N_CORES]
    mesh = Mesh(np.asarray(devices), ("core",))
    n_args = n_params + len(out_names)
    sharded = jax.jit(
        shard_map(
            _body,
            mesh=mesh,
            in_specs=(PartitionSpec("core"),) * n_args,
            out_specs=(PartitionSpec("core"),) * len(out_names),
            check_rep=False,
        ),
        keep_unused=True,
    )
    _COMPILED["exec"] = (sharded, in_names, out_names, out_avals, zero_outs, mesh)
    return _COMPILED["exec"]


def _make_args(in_maps):
    sharded, in_names, out_names, out_avals, zero_outs, mesh = _get_exec()
    concat_in = [
        np.concatenate([np.asarray(in_maps[c][nm]) for c in range(N_CORES)], axis=0)
        for nm in in_names
    ]
    concat_zeros = [
        np.zeros((N_CORES * z.shape[0], *z.shape[1:]), z.dtype) for z in zero_outs
    ]
    return concat_in + concat_zeros


def _run(in_maps):
    sharded, in_names, out_names, out_avals, zero_outs, mesh = _get_exec()
    out_arrs = sharded(*_make_args(in_maps))
    return [
        {
            nm: np.asarray(out_arrs[i]).reshape(N_CORES, *out_avals[i].shape)[c]
            for i, nm in enumerate(out_names)
        }
        for c in range(N_CORES)
    ]


def make_in_maps(x, W, b):
    xpad, wt, bias_mat = _prep_inputs(x, W, b)
    return [
        {"x": np.ascontiguousarray(xpad[i]), "wt": wt, "bias": bias_mat}
        for i in range(N_CORES)
    ]


def kernel(x, W, b):
    results = _run(make_in_maps(x, W, b))
    return np.stack([results[i]["out"] for i in range(N_CORES)], axis=0)
